# revision 15
# baseline (speedup 1.0000x reference)
"""Distributed causal attention (RoPE) kernel for 8 TRN2 NeuronCores.

Problem: B=4, S=2048, dim=2048, H=16 heads, D=128 head dim.
  q,k,v = x @ W{q,k,v}.T (heads), RoPE(q,k), causal softmax(q k^T/sqrt(D)) v,
  out = concat_heads @ Wo.T

Sharding: tensor-parallel over heads — 2 heads per core. Each core computes
qT/kT [d, t] and v [s, e] for its 2 heads, attention in "scoresT" orientation
[key s on partitions, query t free] (exp without max-subtraction: scores are
~N(0,1) so exp cannot overflow; softmax denominator via an all-ones [128,128]
stationary matmul so it lands pre-broadcast), All-to-Alls reshard attention
output from head-shard to row-shard, then a row-local output projection.

Schedule (v3):
 - head-0 attention tq-tiles interleave into the projection blocks of the
   same batch (causal tq n needs only k/v blocks 0..n); each batch's last
   tile slides into the next batch's first block so its exps overlap PE work.
 - PSUM->SBUF casts on Scalar + rotate_half copies on DVE free each PSUM
   bank fast; RoPE multiplies are full-width bf16.
 - scores use wide 2-bank PSUM tiles, one paired EXP per two key tiles.
 - each A2A is split into two 1MB row-half collectives: chunk A (tq0/tq2
   slabs) fires while tq3 still computes, and output-projection units for
   rows [0:512) unblock after chunk A alone.
 - output projection: pass A (head-0 contraction, bf16 partials) interleaves
   into the ACT-bound head-1 attention and the A2A#1 flight; pass B adds the
   head-1 contraction and streams out.
"""

import numpy as np
import ml_dtypes

B, S, DIM = 4, 2048, 2048
H, D = 16, 128
NCORES = 8
HPC = H // NCORES            # heads per core = 2
E = HPC * D                  # per-core inner width = 256
BS = B * S                   # 8192 flattened rows
KT = DIM // 128              # 16 contraction tiles
TQ = 512                     # query tile width
NQ = S // TQ                 # 4 query tiles per (b,h)
NB = S // TQ                 # 4 x-blocks per batch
ROWS = BS // NCORES          # 1024 output rows per core
HR = ROWS // 2               # row-half chunk for split collectives
SCALE = 1.0 / np.sqrt(D)

_CACHE = {}


def _build(causal: bool):
    from concourse import bacc, tile, mybir

    f32 = mybir.dt.float32
    bf16 = mybir.dt.bfloat16
    Exp = mybir.ActivationFunctionType.Exp

    nc = bacc.Bacc(None, target_bir_lowering=False, num_devices=NCORES)

    # host layouts pre-tiled so every DMA is contiguous per partition
    xT_d = nc.dram_tensor("xT", [B * NB, 128, KT, TQ], bf16, kind="ExternalInput")
    wq_d = nc.dram_tensor("wqT", [128, KT, E], bf16, kind="ExternalInput")
    wk_d = nc.dram_tensor("wkT", [128, KT, E], bf16, kind="ExternalInput")
    wv_d = nc.dram_tensor("wvT", [128, KT, E], bf16, kind="ExternalInput")
    wo_d = nc.dram_tensor("woT", [DIM // TQ, 128, 2, KT // 2, TQ], bf16,
                          kind="ExternalInput")
    cos_d = nc.dram_tensor("cosT", [128, BS], bf16, kind="ExternalInput")
    sin_d = nc.dram_tensor("sinT", [128, BS], bf16, kind="ExternalInput")
    msk_d = nc.dram_tensor("mask0", [128, TQ], bf16, kind="ExternalInput")
    out_d = nc.dram_tensor("out", [ROWS, DIM], bf16, kind="ExternalOutput")

    with tile.TileContext(nc) as tc:
        with (
            tc.tile_pool(name="const", bufs=1) as constp,
            tc.tile_pool(name="dram", bufs=1, space="DRAM") as dramp,
        ):
            # [head][chunk]: chunk 0 holds rows [0:512) of every dest core
            # (tq0/tq2 slabs), chunk 1 rows [512:1024) (tq1/tq3)
            a2a_in = [[dramp.tile([NCORES, 128, HR], bf16, name=f"a2ai{h}{c}")
                       for c in range(2)] for h in range(HPC)]
            a2a_out = [[dramp.tile([NCORES, 128, HR], bf16, name=f"a2ao{h}{c}")
                        for c in range(2)] for h in range(HPC)]

            warm_in = dramp.tile([NCORES, 128, 8], bf16, name="warmi")
            warm_out = dramp.tile([NCORES, 128, 8], bf16, name="warmo")
            nc.gpsimd.collective_compute(
                "AllToAll", mybir.AluOpType.bypass,
                replica_groups=[list(range(NCORES))],
                ins=[warm_in[:].opt()], outs=[warm_out[:].opt()],
            )
            ones_col = constp.tile([128, 128], bf16)
            nc.gpsimd.memset(ones_col[:], 1.0)
            if causal:
                msk_sb = constp.tile([128, TQ], bf16)
                nc.scalar.dma_start(msk_sb[:], msk_d[:])

            def a2a(h, c):
                nc.gpsimd.collective_compute(
                    "AllToAll", mybir.AluOpType.bypass,
                    replica_groups=[list(range(NCORES))],
                    ins=[a2a_in[h][c][:].opt()], outs=[a2a_out[h][c][:].opt()],
                )

            with (
                tc.tile_pool(name="qkv", bufs=4) as qkvp,
                tc.tile_pool(name="ex", bufs=3) as exp_pool,
                tc.tile_pool(name="att", bufs=2) as ap,
                tc.tile_pool(name="ps1", bufs=2, space="PSUM") as pp1,
                tc.tile_pool(name="ps2", bufs=1, space="PSUM") as pp2,
            ):

                def att_tq(b, h, tq, qb, kb, vb):
                    """One query tile: wide score tiles, paired exps, one-pair
                    lookahead so the PE never waits on the ACT exp latency."""
                    t0 = tq * TQ
                    jmax = (tq + 1) * (TQ // 128) if causal else S // 128
                    npair = jmax // 2
                    av = pp2.tile([128, TQ], f32, tag="av", bufs=1)
                    cs = pp2.tile([128, TQ], f32, tag="cs", bufs=1)
                    pend = []

                    def flush_one():
                        for (buf, lo, hi, off, j) in pend.pop(0):
                            nc.tensor.matmul(
                                cs[:, off:TQ], ones_col[:], buf[:, lo:hi],
                                start=(j == 0), stop=(j == jmax - 1),
                            )
                            nc.tensor.matmul(
                                av[:, off:TQ], vb[:, j, h * 128:(h + 1) * 128],
                                buf[:, lo:hi],
                                start=(j == 0), stop=(j == jmax - 1),
                            )

                    for jp in range(npair):
                        scw = pp2.tile([128, 2 * TQ], f32, tag="sc", bufs=2)
                        info = []
                        for jj in range(2):
                            j = 2 * jp + jj
                            s0 = j * 128
                            diag = causal and j >= jmax - 4
                            off = 128 * (j - (jmax - 4)) if diag else 0
                            nc.tensor.matmul(
                                scw[:, jj * TQ + off:(jj + 1) * TQ],
                                kb[:, h, s0:s0 + 128],
                                qb[:, h, t0 + off:t0 + TQ],
                                start=True, stop=True,
                            )
                            info.append((j, jj, off, diag))
                        off0 = info[0][2]
                        exw = exp_pool.tile([128, 2 * TQ], bf16, tag="ex", bufs=4)
                        nc.scalar.activation(exw[:, off0:2 * TQ],
                                             scw[:, off0:2 * TQ], Exp,
                                             scale=float(SCALE))
                        srcs = []
                        exm = None
                        for (j, jj, off, diag) in info:
                            lo, hi = jj * TQ + off, (jj + 1) * TQ
                            if diag:
                                if exm is None:
                                    exm = exp_pool.tile([128, 2 * TQ], bf16,
                                                        tag="exm", bufs=2)
                                nc.vector.tensor_mul(exm[:, lo:hi],
                                                     exw[:, lo:hi],
                                                     msk_sb[:, 0:TQ - off])
                                srcs.append((exm, lo, hi, off, j))
                            else:
                                srcs.append((exw, lo, hi, off, j))
                        pend.append(srcs)
                        if len(pend) > 1:
                            flush_one()
                    while pend:
                        flush_one()

                    rec = ap.tile([128, TQ], f32, tag="rec", bufs=1)
                    nc.vector.reciprocal_approx_fast(rec[:], cs[:])
                    ot = ap.tile([128, TQ], bf16, tag="ot", bufs=2)
                    nc.vector.tensor_mul(ot[:], av[:], rec[:])
                    # slab (b,tq) -> chunk tq%2, dest 2b + (tq>=2), full 512
                    nc.sync.dma_start(
                        a2a_in[h][tq % 2][2 * b + (tq // 2), :, :], ot[:]
                    )

                batches = []
                with (
                    tc.tile_pool(name="w1", bufs=1) as w1p,
                    tc.tile_pool(name="xblk", bufs=2) as xp,
                    tc.tile_pool(name="cs", bufs=3) as cp,
                    tc.tile_pool(name="rope", bufs=2) as rp,
                ):
                    # startup weight loads spread across engine queues
                    wq_sb = w1p.tile([128, KT, E], bf16, name="wq")
                    wk_sb = w1p.tile([128, KT, E], bf16, name="wk")
                    wv_sb = w1p.tile([128, KT, E], bf16, name="wv")
                    nc.sync.dma_start(wq_sb[:, 0:KT // 2, :], wq_d[:, 0:KT // 2, :])
                    nc.scalar.dma_start(wq_sb[:, KT // 2:, :], wq_d[:, KT // 2:, :])
                    nc.scalar.dma_start(wk_sb[:], wk_d[:])
                    nc.scalar.dma_start(wv_sb[:], wv_d[:])

                    def p1_block(b, n, qb, kb, vb):
                        c0 = n * TQ
                        g0 = b * S + c0
                        xblk = xp.tile([128, KT, TQ], bf16, tag="x")
                        if b == 0 and n == 0:
                            nc.gpsimd.dma_start(xblk[:, 0:KT // 2, :],
                                                xT_d[0, :, 0:KT // 2, :])
                            nc.sync.dma_start(xblk[:, KT // 2:, :],
                                              xT_d[0, :, KT // 2:, :])
                        else:
                            eng = nc.sync if n % 2 == 0 else nc.gpsimd
                            eng.dma_start(xblk[:], xT_d[b * NB + n])
                        cos_b = cp.tile([128, TQ], bf16, tag="cos")
                        sin_b = cp.tile([128, TQ], bf16, tag="sin")
                        nc.gpsimd.dma_start(cos_b[:], cos_d[:, g0:g0 + TQ])
                        nc.gpsimd.dma_start(sin_b[:], sin_d[:, g0:g0 + TQ])

                        for w_sb, dst in ((wq_sb, qb), (wk_sb, kb)):
                            for h in range(HPC):
                                ps = pp1.tile([128, TQ], f32, tag="qk", bufs=2)
                                for k in range(KT):
                                    nc.tensor.matmul(
                                        ps[:], w_sb[:, k, h * 128:(h + 1) * 128],
                                        xblk[:, k, :],
                                        start=(k == 0), stop=(k == KT - 1),
                                    )
                                # straight cast on ACT, rotate_half halves on
                                # DVE (PSUM source may cross partitions), so
                                # the PSUM bank frees after 3 parallel ops
                                pbf = rp.tile([128, TQ], bf16, tag="pbf", bufs=3)
                                prot = rp.tile([128, TQ], bf16, tag="prot", bufs=3)
                                nc.scalar.copy(pbf[:], ps[:])
                                nc.vector.tensor_copy(prot[0:64, :], ps[64:128, :])
                                nc.vector.tensor_copy(prot[64:128, :], ps[0:64, :])
                                t0_ = rp.tile([128, TQ], bf16, tag="t0", bufs=2)
                                nc.vector.tensor_mul(t0_[:], prot[:], sin_b[:])
                                t1_ = rp.tile([128, TQ], bf16, tag="t1", bufs=2)
                                nc.vector.tensor_mul(t1_[:], pbf[:], cos_b[:])
                                nc.vector.tensor_add(dst[:, h, c0:c0 + TQ],
                                                     t0_[:], t1_[:])

                        for ss in range(TQ // 128):
                            vps = pp1.tile([128, TQ], f32, tag="qk", bufs=2)
                            for k in range(KT):
                                nc.tensor.matmul(
                                    vps[:, 0:E],
                                    xblk[:, k, ss * 128:(ss + 1) * 128],
                                    wv_sb[:, k, :],
                                    start=(k == 0), stop=(k == KT - 1),
                                )
                            nc.scalar.copy(vb[:, n * 4 + ss, :], vps[:, 0:E])

                    for b in range(B):
                        qb = qkvp.tile([128, HPC, S], bf16, tag="q", name=f"q{b}")
                        kb = qkvp.tile([128, HPC, S], bf16, tag="k", name=f"k{b}")
                        vb = qkvp.tile([128, S // 128, E], bf16, tag="v",
                                       name=f"v{b}")
                        for n in range(NB):
                            p1_block(b, n, qb, kb, vb)
                            if not causal:
                                continue
                            # both heads' attention tiles ride inside the
                            # PE-dense projection stream; batch b3's head-1
                            # stays back to cover the collective window
                            if b > 0 and n == 0:
                                att_tq(b - 1, 0, 3, *batches[b - 1])
                                att_tq(b - 1, 1, 3, *batches[b - 1])
                            elif n >= 1:
                                att_tq(b, 0, n - 1, qb, kb, vb)
                                if b < B - 1:
                                    att_tq(b, 1, n - 1, qb, kb, vb)
                        if causal and b == B - 1:
                            att_tq(b - 1, 1, 3, *batches[b - 1])
                        batches.append((qb, kb, vb))

                # scope1 closed: xblk/cos/rope/weights SBUF freed for outproj
                with (
                    tc.tile_pool(name="attn_in", bufs=1) as atp,
                    tc.tile_pool(name="wo", bufs=3) as wop,
                    tc.tile_pool(name="res", bufs=2) as resp,
                ):
                    at_sb = atp.tile([128, KT, ROWS], bf16)  # [e%128, e//128, t]

                    def gather(h, c):
                        for i in range(NCORES):
                            nc.sync.dma_start(
                                at_sb[:, 2 * i + h, c * HR:(c + 1) * HR],
                                a2a_out[h][c][i],
                            )

                    def opsum(k, four=True):
                        m = k % 4 if four else k % 2
                        if m == 0:
                            t = pp1.tile([128, TQ], f32, tag="qk", bufs=2,
                                         name=f"op{k}")
                            return t[:]
                        if m == 1:
                            t = pp2.tile([128, 2 * TQ], f32, tag="sc", bufs=2,
                                         name=f"op{k}")
                            return t[:, 0:TQ]
                        if m == 2:
                            t = pp2.tile([128, TQ], f32, tag="av", bufs=1,
                                         name=f"op{k}")
                            return t[:]
                        t = pp2.tile([128, TQ], f32, tag="cs", bufs=1,
                                     name=f"op{k}")
                        return t[:]

                    att_done = [False]
                    pas = {}
                    # pass-A units: rows [0:512) first (need chunk A only)
                    aunits = ([(f, tt) for f in range(DIM // TQ)
                               for tt in range(4)] +
                              [(f, tt) for f in range(DIM // TQ)
                               for tt in range(4, 8)])
                    ai = [0]
                    wo_t = {}

                    def wo_tile(f, half):
                        key = (f, half, ai[0] // 16)
                        if key not in wo_t:
                            t = wop.tile([128, KT // 2, TQ], bf16, tag="wo",
                                         name=f"wo{f}h{half}_{ai[0] // 16}")
                            nc.sync.dma_start(t[:], wo_d[f, :, half])
                            wo_t[key] = t
                        return wo_t[key]

                    def issue_aunit(kcnt):
                        while kcnt > 0 and ai[0] < len(aunits):
                            f, tt = aunits[ai[0]]
                            wt = wo_tile(f, 0)
                            ai[0] += 1
                            kcnt -= 1
                            # rotate over 6 PSUM banks (av/cs join once
                            # attention is done) so the PE never waits on the
                            # DVE draining a previous unit's accumulator
                            dstv = opsum(ai[0], four=att_done[0])
                            for ki in range(KT // 2):
                                nc.tensor.matmul(
                                    dstv,
                                    at_sb[:, 2 * ki, tt * 128:(tt + 1) * 128],
                                    wt[:, ki, :],
                                    start=(ki == 0), stop=(ki == KT // 2 - 1),
                                )
                            pa = resp.tile([128, TQ], bf16, tag="pa",
                                           name=f"pa{f}_{tt}", bufs=32)
                            nc.vector.tensor_copy(pa[:], dstv)
                            pas[(f, tt)] = pa

                    if causal:
                        qb, kb, vb = batches[B - 1]
                        a2a(0, 0)
                        att_tq(B - 1, 0, 3, qb, kb, vb)
                        a2a(0, 1)
                        gather(0, 0)
                        att_tq(B - 1, 1, 0, qb, kb, vb)
                        att_tq(B - 1, 1, 1, qb, kb, vb)
                        gather(0, 1)
                        att_tq(B - 1, 1, 2, qb, kb, vb)
                        issue_aunit(3)
                        a2a(1, 0)
                        att_tq(B - 1, 1, 3, qb, kb, vb)
                        issue_aunit(3)
                        a2a(1, 1)
                        att_done[0] = True
                        issue_aunit(len(aunits))
                        gather(1, 0)
                        gather(1, 1)
                    else:
                        for b in range(B):
                            qb, kb, vb = batches[b]
                            for tq in range(NQ):
                                att_tq(b, 0, tq, qb, kb, vb)
                                att_tq(b, 1, tq, qb, kb, vb)
                        a2a(0, 0)
                        a2a(0, 1)
                        gather(0, 0)
                        gather(0, 1)
                        a2a(1, 0)
                        a2a(1, 1)
                        att_done[0] = True
                        issue_aunit(len(aunits))
                        gather(1, 0)
                        gather(1, 1)

                    # pass B: head-1 contraction + partial add, rows [0:512)
                    # first (they need only A2A#1 chunk A), stream out
                    wo_b = {}
                    for thalf in range(2):
                        for f in range(DIM // TQ):
                            key = (f, thalf)
                            wo_b[key] = wop.tile([128, KT // 2, TQ], bf16,
                                                 tag="wo", name=f"woB{f}_{thalf}")
                            nc.scalar.dma_start(wo_b[key][:], wo_d[f, :, 1])
                            for tt in range(4 * thalf, 4 * thalf + 4):
                                dstv = opsum(4 * thalf + tt)
                                for ki in range(KT // 2):
                                    nc.tensor.matmul(
                                        dstv,
                                        at_sb[:, 2 * ki + 1,
                                              tt * 128:(tt + 1) * 128],
                                        wo_b[key][:, ki, :],
                                        start=(ki == 0),
                                        stop=(ki == KT // 2 - 1),
                                    )
                                res = resp.tile([128, TQ], bf16, tag="res",
                                                bufs=4)
                                nc.vector.tensor_add(res[:], dstv,
                                                     pas[(f, tt)][:])
                                oeng = nc.sync if tt % 2 == 0 else nc.gpsimd
                                oeng.dma_start(
                                    out_d[tt * 128:(tt + 1) * 128,
                                          f * TQ:(f + 1) * TQ],
                                    res[:],
                                )

    nc.compile()
    return nc


def _prep_inputs(x, Wq, Wk, Wv, Wo, causal):
    bf16 = ml_dtypes.bfloat16
    xT = np.ascontiguousarray(x.reshape(BS, DIM).T).astype(bf16)  # [dim, BS]
    # [block, p, ktile, t] so each block DMA is contiguous per partition
    xTt = np.ascontiguousarray(
        xT.reshape(KT, 128, B * NB, TQ).transpose(2, 1, 0, 3))
    woT = np.ascontiguousarray(Wo.T).astype(bf16)                 # [e, f]
    # [f, p, half, ki, t]: e = (2*ki+half)*128 + p
    woTt = np.ascontiguousarray(
        woT.reshape(KT // 2, 2, 128, DIM // TQ, TQ).transpose(3, 2, 1, 0, 4))

    # RoPE tables in [d, pos] layout, tiled over batches; sin pre-signed for
    # rotate_half (rows 0:64 multiply the shifted-up half, hence negative).
    inv_freq = 1.0 / (10000.0 ** (np.arange(0, D, 2, dtype=np.float64) / D))
    t = np.arange(S, dtype=np.float64)
    freqs = np.outer(t, inv_freq)                      # [S, 64]
    emb = np.concatenate([freqs, freqs], axis=-1)      # [S, D]
    cosT = np.tile(np.cos(emb).T.astype(np.float32), (1, B)).astype(bf16)
    sinN = np.sin(emb).T.astype(np.float32)
    sinN[0:64] *= -1.0
    sinT = np.tile(sinN, (1, B)).astype(bf16)

    ii = np.arange(128)[:, None]
    jj = np.arange(TQ)[None, :]
    mask0 = (jj >= ii).astype(bf16)

    in_maps = []
    for c in range(NCORES):
        e0, e1 = c * E, (c + 1) * E
        in_maps.append({
            "xT": xTt,
            "wqT": np.ascontiguousarray(
                Wq[e0:e1].T.astype(bf16).reshape(KT, 128, E).transpose(1, 0, 2)),
            "wkT": np.ascontiguousarray(
                Wk[e0:e1].T.astype(bf16).reshape(KT, 128, E).transpose(1, 0, 2)),
            "wvT": np.ascontiguousarray(
                Wv[e0:e1].T.astype(bf16).reshape(KT, 128, E).transpose(1, 0, 2)),
            "woT": woTt,
            "cosT": cosT,
            "sinT": sinT,
            "mask0": mask0,
        })
    return in_maps


def kernel(x, Wq, Wk, Wv, Wo, mask, _trace=False):
    from concourse.bass_utils import run_bass_kernel_spmd

    m = np.asarray(mask)
    causal = not bool(m.reshape(m.shape[-2], m.shape[-1])[0, -1])

    if causal not in _CACHE:
        _CACHE[causal] = _build(causal)
    nc = _CACHE[causal]

    in_maps = _prep_inputs(np.asarray(x), np.asarray(Wq), np.asarray(Wk),
                           np.asarray(Wv), np.asarray(Wo), causal)
    res = run_bass_kernel_spmd(nc, in_maps, core_ids=list(range(NCORES)),
                               trace=_trace)
    full = np.concatenate([np.asarray(res.results[c]["out"])
                           for c in range(NCORES)], axis=0)
    out = full.reshape(B, S, DIM).astype(np.float32)
    if _trace:
        return out, res
    return out
